# revision 5
# baseline (speedup 1.0000x reference)
"""Trainium2 Bass kernel for the two-level Haar-DWT detail (L1) loss.

Strategy (pure data parallel over batch, 8 NeuronCores):
  - Each core gets 4 of the 32 batch images (both `output` and `target`),
    viewed as a [6144, 512] row matrix (batch*chan*height rows).
  - The loss is linear until the |.| per band, and the normalize step
    ((x+1)/2 applied to both inputs) only scales the difference
    d = output - target by 0.5, which is folded into host-side scaling.
  - Per 128-row tile, the TensorEngine (fp32r / TF32 mode, full rate at
    N>=256) computes subtract + row pair-combine + column pair-combine in
    one pass: 8 accumulating matmuls with +-1 weights on even/odd-column
    strided views of o and t produce the stacked level-1 subbands
    2*(LL1|LH1) and 2*(HL1|HH1) directly in PSUM.
  - The ScalarEngine does fused abs + per-partition accumulate on each
    band block.  The VectorEngine copies the LL1 rows to SBUF (rounding
    to fp32r) and forms the level-2 column combines; one more matmul
    makes the level-2 bands, reduced with fused-abs on the VectorEngine.
  - Each core emits a [128, 4] tensor of per-partition abs-sums; the host
    combines them into the scalar loss (float64 accumulate).
"""

import numpy as np

B, C, H, W = 32, 3, 512, 512
N_CORES = 8
B_PER_CORE = B // N_CORES
ROWS = B_PER_CORE * C * H  # 6144
COLS = W  # 512
NT = ROWS // 128  # 48 tiles per core

_CACHE = {}


def _make_weights():
    # w1[k, m]: row pair-combine. m<64: +1 at rows 2m, 2m+1 (pair sum);
    # m=64+mm: -1 at 2mm, +1 at 2mm+1 (pair diff).
    w1p = np.zeros((128, 128), np.float32)
    for m in range(64):
        w1p[2 * m, m] = 1.0
        w1p[2 * m + 1, m] = 1.0
        w1p[2 * m, 64 + m] = -1.0
        w1p[2 * m + 1, 64 + m] = 1.0
    w1n = -w1p
    # w2sd[k, m] (64x64): LL1-row pair-combine. m<32: pair sum; m=32+mm:
    # pair diff.
    w2sd = np.zeros((64, 64), np.float32)
    for m in range(32):
        w2sd[2 * m, m] = 1.0
        w2sd[2 * m + 1, m] = 1.0
        w2sd[2 * m, 32 + m] = -1.0
        w2sd[2 * m + 1, 32 + m] = 1.0
    return w1p, w1n, w2sd


def _build_bass():
    from contextlib import ExitStack

    import concourse.bacc as bacc
    import concourse.mybir as mybir
    import concourse.tile as tile

    F32 = mybir.dt.float32
    F32R = mybir.dt.float32r
    X = mybir.AxisListType.X
    ADD = mybir.AluOpType.add
    ABS = mybir.ActivationFunctionType.Abs

    nc = bacc.Bacc("TRN2", target_bir_lowering=False, debug=False,
                   num_devices=N_CORES)
    o_d = nc.dram_tensor("o", [ROWS, COLS], F32R, kind="ExternalInput").ap()
    t_d = nc.dram_tensor("t", [ROWS, COLS], F32R, kind="ExternalInput").ap()
    w1p_d = nc.dram_tensor("w1p", [128, 128], F32R, kind="ExternalInput").ap()
    w1n_d = nc.dram_tensor("w1n", [128, 128], F32R, kind="ExternalInput").ap()
    w2sd_d = nc.dram_tensor("w2sd", [64, 64], F32R, kind="ExternalInput").ap()
    res_d = nc.dram_tensor("res", [128, 4], F32, kind="ExternalOutput").ap()

    with tile.TileContext(nc) as tc, ExitStack() as ctx:
        consts = ctx.enter_context(tc.tile_pool(name="consts", bufs=1))
        loads = ctx.enter_context(tc.tile_pool(name="loads", bufs=4))
        bands = ctx.enter_context(tc.tile_pool(name="bands", bufs=3))
        absout = ctx.enter_context(tc.tile_pool(name="absout", bufs=2))
        psA = ctx.enter_context(tc.tile_pool(name="psA", bufs=2, space="PSUM"))
        psB = ctx.enter_context(tc.tile_pool(name="psB", bufs=2, space="PSUM"))
        accp = ctx.enter_context(tc.tile_pool(name="accp", bufs=1))

        w1p_t = consts.tile([128, 128], F32R)
        w1n_t = consts.tile([128, 128], F32R)
        w2sd_t = consts.tile([64, 64], F32R)
        nc.sync.dma_start(w1p_t[:], w1p_d)
        nc.sync.dma_start(w1n_t[:], w1n_d)
        nc.sync.dma_start(w2sd_t[:], w2sd_d)

        acc1 = accp.tile([128, NT], F32)
        acc2 = accp.tile([128, NT], F32)
        acc34 = accp.tile([64, NT // 2], F32)

        sbLL = None
        for it in range(NT):
            half = it % 2
            o_t = loads.tile([128, COLS], F32R, tag="o_t")
            t_t = loads.tile([128, COLS], F32R, tag="t_t")
            nc.sync.dma_start(o_t[:], o_d[it * 128:(it + 1) * 128, :])
            nc.sync.dma_start(t_t[:], t_d[it * 128:(it + 1) * 128, :])

            oe, oo = o_t[:, 0:COLS:2], o_t[:, 1:COLS:2]
            te, to = t_t[:, 0:COLS:2], t_t[:, 1:COLS:2]

            # psumS = row-combine(col-pair-sum(d)) = 2*(LL1|LH1) of d=o-t
            # psumD = row-combine(col-pair-diff(d)) = 2*(HL1|HH1)
            # Matmuls ordered to group equal weights (fewer PE reloads).
            psumS = psA.tile([128, COLS // 2], F32, tag="psumS")
            psumD = psA.tile([128, COLS // 2], F32, tag="psumD")
            mm = nc.tensor.matmul
            mm(psumS[:], lhsT=w1p_t[:], rhs=oe, start=True, stop=False)
            mm(psumS[:], lhsT=w1p_t[:], rhs=oo, start=False, stop=False)
            mm(psumD[:], lhsT=w1p_t[:], rhs=oo, start=True, stop=False)
            mm(psumD[:], lhsT=w1p_t[:], rhs=te, start=False, stop=False)
            mm(psumD[:], lhsT=w1n_t[:], rhs=oe, start=False, stop=False)
            mm(psumD[:], lhsT=w1n_t[:], rhs=to, start=False, stop=True)
            mm(psumS[:], lhsT=w1n_t[:], rhs=te, start=False, stop=False)
            mm(psumS[:], lhsT=w1n_t[:], rhs=to, start=False, stop=True)

            # ScalarEngine: fused abs + per-partition sum of both L1 blocks.
            ab1 = absout.tile([128, COLS // 2], F32, tag="ab1")
            ab2 = absout.tile([128, COLS // 2], F32, tag="ab2")
            nc.scalar.activation(ab1[:], psumS[:], ABS,
                                 accum_out=acc1[:, it:it + 1])
            nc.scalar.activation(ab2[:], psumD[:], ABS,
                                 accum_out=acc2[:, it:it + 1])

            # Level 2 (on pairs of tiles): copy LL1 rows (parts 0-63 of
            # psumS) to SBUF, fp32r-rounded, two tiles side by side.
            if half == 0:
                sbLL = bands.tile([64, 512], F32R, tag="sbLL")
            nc.vector.tensor_copy(sbLL[:, half * 256:(half + 1) * 256],
                                  psumS[0:64, :])
            if half == 1:
                pr = it // 2
                # rhs2 = [cd2 | cs2]: level-2 column pair diff/sum.
                rhs2 = bands.tile([64, 512], F32R, tag="rhs2")
                nc.vector.tensor_sub(rhs2[:, 0:256],
                                     sbLL[:, 1:512:2], sbLL[:, 0:512:2])
                nc.vector.tensor_add(rhs2[:, 256:512],
                                     sbLL[:, 0:512:2], sbLL[:, 1:512:2])
                # psumC rows 0-31: [4*HL2 (A|B) | junk]; rows 32-63:
                # [4*HH2 (A|B) | 4*LH2 (A|B)]
                psumC = psB.tile([64, 512], F32)
                mm(psumC[:], lhsT=w2sd_t[:], rhs=rhs2[:],
                   start=True, stop=True)
                nc.vector.tensor_reduce(acc34[32:64, pr:pr + 1],
                                        psumC[32:64, :], axis=X, op=ADD,
                                        apply_absolute_value=True)
                nc.vector.tensor_reduce(acc34[0:32, pr:pr + 1],
                                        psumC[0:32, 0:256], axis=X, op=ADD,
                                        apply_absolute_value=True)

        res_t = accp.tile([128, 4], F32)
        nc.vector.memset(res_t[:], 0.0)
        nc.vector.tensor_reduce(res_t[:, 0:1], acc1[:], axis=X, op=ADD)
        nc.vector.tensor_reduce(res_t[:, 1:2], acc2[:], axis=X, op=ADD)
        nc.vector.tensor_reduce(res_t[0:64, 2:3], acc34[:], axis=X, op=ADD)
        nc.sync.dma_start(res_d, res_t[:])

    nc.compile()
    return nc


def _get_bass():
    if "nc" not in _CACHE:
        _CACHE["nc"] = _build_bass()
    return _CACHE["nc"]


def _numpy_reference(output, target):
    """Full-precision fallback (only for the never-hit mixed-normalize case)."""
    o = output.astype(np.float64)
    t = target.astype(np.float64)
    if o.min() < 0:
        o = (o + 1.0) * 0.5
    if t.min() < 0:
        t = (t + 1.0) * 0.5

    def dwt(x):
        a = x[:, :, 0::2, 0::2]
        b = x[:, :, 0::2, 1::2]
        c = x[:, :, 1::2, 0::2]
        d = x[:, :, 1::2, 1::2]
        return (0.5 * (a + b + c + d), 0.5 * (-a - b + c + d),
                0.5 * (-a + b - c + d), 0.5 * (a - b - c + d))

    ll_o, lh_o, hl_o, hh_o = dwt(o)
    ll_t, lh_t, hl_t, hh_t = dwt(t)
    tot = (np.abs(lh_o - lh_t).mean() + np.abs(hl_o - hl_t).mean()
           + np.abs(hh_o - hh_t).mean() + 0.1 * np.abs(ll_o - ll_t).mean())
    _, lh2_o, hl2_o, hh2_o = dwt(ll_o)
    _, lh2_t, hl2_t, hh2_t = dwt(ll_t)
    tot += 0.5 * (np.abs(lh2_o - lh2_t).mean() + np.abs(hl2_o - hl2_t).mean()
                  + np.abs(hh2_o - hh2_t).mean())
    return np.float32(tot)


def _run_device(o, t, trace=False):
    """Shard [32,3,512,512] f32 arrays over 8 cores and run the Bass NEFF."""
    from concourse.bass_utils import run_bass_kernel_spmd

    nc = _get_bass()
    w1p, w1n, w2sd = _make_weights()
    in_maps = []
    for c in range(N_CORES):
        sl = slice(c * B_PER_CORE, (c + 1) * B_PER_CORE)
        in_maps.append({
            "o": o[sl].reshape(ROWS, COLS),
            "t": t[sl].reshape(ROWS, COLS),
            "w1p": w1p, "w1n": w1n, "w2sd": w2sd,
        })
    res = run_bass_kernel_spmd(nc, in_maps, core_ids=list(range(N_CORES)),
                               trace=trace)
    _CACHE["last_result"] = res
    return res


def combine(results, both_norm=True):
    """Combine per-core [128, 4] abs-sum tensors into the scalar loss."""
    S_LL1 = S_LH1 = S_HL1 = S_HH1 = S_L2 = 0.0
    for r in results:
        v = r.astype(np.float64)
        S_LL1 += v[0:64, 0].sum()
        S_LH1 += v[64:128, 0].sum()
        S_HL1 += v[0:64, 1].sum()
        S_HH1 += v[64:128, 1].sum()
        S_L2 += v[0:64, 2].sum()  # |HL2|+|HH2|+|LH2| at 4x scale

    n1 = float(B * C * (H // 2) * (W // 2))
    n2 = float(B * C * (H // 4) * (W // 4))
    # Device L1 bands carry 2x scale, L2 bands 4x; both-normalized inputs
    # add 0.5 on d (and another 0.5 on LL1 before level 2).
    l1_scale, l2_scale = (4.0, 8.0) if both_norm else (2.0, 4.0)
    total = ((S_LH1 + S_HL1 + S_HH1 + 0.1 * S_LL1) / (l1_scale * n1)
             + 0.5 * S_L2 / (l2_scale * n2))
    return np.float32(total)


def kernel(output, target):
    o = np.ascontiguousarray(np.asarray(output, dtype=np.float32))
    t = np.ascontiguousarray(np.asarray(target, dtype=np.float32))
    o_norm = bool(o.min() < 0.0)
    t_norm = bool(t.min() < 0.0)
    if o_norm != t_norm:
        # Normalization applied to only one input: the difference is no
        # longer a pure scale of o - t.  Practically unreachable for the
        # randn inputs this problem uses.
        return _numpy_reference(o, t)

    results = [r["res"] for r in _run_device(o, t).results]
    return combine(results, both_norm=o_norm)


# revision 6
# speedup vs baseline: 1.1031x; 1.1031x over previous
"""Trainium2 Bass kernel for the two-level Haar-DWT detail (L1) loss.

Strategy (pure data parallel over batch, 8 NeuronCores):
  - Each core gets 4 of the 32 batch images (both `output` and `target`),
    viewed as a [6144, 512] row matrix; 48 row-tiles of [128, 512].
  - The loss is linear until the per-band |.|; the (x+1)/2 normalization
    of both inputs only scales d = output - target by 0.5 (host-folded).
  - Per tile, the VectorEngine computes d = o - t in bf16 with a
    phase-major column layout (columns grouped by col%4) so the level-1/2
    column pair-combines run at the bf16 2x DVE rate on contiguous halves.
    Column permutations are free: every band ends in an abs-sum.
  - The TensorEngine (bf16, 1 cycle/row) folds the row pair-combines:
    psum <- W^T cs / W^T cd give the four level-1 bands; the 0.1 LL
    weight is baked into W (q = bf16(0.1)).  Two more small matmuls give
    the level-2 bands from the level-2 column combines.
  - All six band blocks of a PAIR of tiles land in one 3-bank PSUM
    region [128, 1280]; a single ScalarEngine Abs-activation with
    accum_out produces the per-partition abs-sum. The relative band
    weights are arranged so the host just sums everything.
  - Each core emits [128, 4]; host combines in float64.
"""

import numpy as np

B, C, H, W = 32, 3, 512, 512
N_CORES = 8
B_PER_CORE = B // N_CORES
ROWS = B_PER_CORE * C * H  # 6144
COLS = W  # 512
NB = 4  # row-blocks per DMA super-tile (1 MiB loads)
NT = ROWS // 128  # 48 tiles per core
NG = NT // NB  # 12 super-tiles
NP = NT // 2  # 24 tile-pairs

_CACHE = {}


def _make_weights():
    import ml_dtypes
    q = ml_dtypes.bfloat16(0.1)  # LL1 loss weight, baked into W1q
    # w1q[k, m]: row pair-combine for the S (col-sum) path.
    # m<64: +q at rows 2m, 2m+1 (pair sum -> LL1, pre-weighted);
    # m=64+mm: -1/+1 at rows 2mm, 2mm+1 (pair diff -> LH1).
    w1q = np.zeros((128, 128), ml_dtypes.bfloat16)
    w1 = np.zeros((128, 128), ml_dtypes.bfloat16)
    for m in range(64):
        w1q[2 * m, m] = q
        w1q[2 * m + 1, m] = q
        w1q[2 * m, 64 + m] = -1.0
        w1q[2 * m + 1, 64 + m] = 1.0
        # plain +-1 for the D (col-diff) path: HL1 | HH1
        w1[2 * m, m] = 1.0
        w1[2 * m + 1, m] = 1.0
        w1[2 * m, 64 + m] = -1.0
        w1[2 * m + 1, 64 + m] = 1.0
    # w24[k, m]: 4-row combines for level 2. m<32: sum of rows 4m..4m+3
    # (-> HL2 from cd2); m=32+mm: -,-,+,+ diff (-> HH2 from cd2).
    w24 = np.zeros((128, 64), ml_dtypes.bfloat16)
    # w24dp: m<32: the 4-row diff (-> LH2 from cs2); m>=32: zero (pads
    # psum rows 32-63 with exact zeros so one abs-sum can span them).
    w24dp = np.zeros((128, 64), ml_dtypes.bfloat16)
    for m in range(32):
        for r in range(4):
            w24[4 * m + r, m] = 1.0
            w24[4 * m + r, 32 + m] = -1.0 if r < 2 else 1.0
            w24dp[4 * m + r, m] = -1.0 if r < 2 else 1.0
    return w1q, w1, w24, w24dp


def _build_bass():
    from contextlib import ExitStack

    import concourse.bacc as bacc
    import concourse.mybir as mybir
    import concourse.tile as tile

    F32 = mybir.dt.float32
    BF16 = mybir.dt.bfloat16
    X = mybir.AxisListType.X
    ADD = mybir.AluOpType.add
    ABS = mybir.ActivationFunctionType.Abs

    nc = bacc.Bacc("TRN2", target_bir_lowering=False, debug=False,
                   num_devices=N_CORES)
    o_d = nc.dram_tensor("o", [ROWS, COLS], F32, kind="ExternalInput").ap()
    t_d = nc.dram_tensor("t", [ROWS, COLS], F32, kind="ExternalInput").ap()
    w1q_d = nc.dram_tensor("w1q", [128, 128], BF16, kind="ExternalInput").ap()
    w1_d = nc.dram_tensor("w1", [128, 128], BF16, kind="ExternalInput").ap()
    w24_d = nc.dram_tensor("w24", [128, 64], BF16, kind="ExternalInput").ap()
    w24dp_d = nc.dram_tensor("w24dp", [128, 64], BF16,
                             kind="ExternalInput").ap()
    res_d = nc.dram_tensor("res", [128, 4], F32, kind="ExternalOutput").ap()

    # DRAM view for 1 MiB loads: [supertile, 128, block, col]
    o_v = o_d.rearrange("(g b p) c -> g p b c", b=NB, p=128)
    t_v = t_d.rearrange("(g b p) c -> g p b c", b=NB, p=128)

    with tile.TileContext(nc) as tc, ExitStack() as ctx:
        consts = ctx.enter_context(tc.tile_pool(name="consts", bufs=1))
        loads = ctx.enter_context(tc.tile_pool(name="loads", bufs=2))
        bands = ctx.enter_context(tc.tile_pool(name="bands", bufs=3))
        absout = ctx.enter_context(tc.tile_pool(name="absout", bufs=2))
        psP = ctx.enter_context(tc.tile_pool(name="psP", bufs=2, space="PSUM"))
        accp = ctx.enter_context(tc.tile_pool(name="accp", bufs=1))

        w1q_t = consts.tile([128, 128], BF16)
        w1_t = consts.tile([128, 128], BF16)
        w24_t = consts.tile([128, 64], BF16)
        w24dp_t = consts.tile([128, 64], BF16)
        nc.sync.dma_start(w1q_t[:], w1q_d)
        nc.sync.dma_start(w1_t[:], w1_d)
        nc.sync.dma_start(w24_t[:], w24_d)
        nc.sync.dma_start(w24dp_t[:], w24dp_d)

        acc = accp.tile([128, NP], F32)
        mm = nc.tensor.matmul

        for g in range(NG):
            o_t = loads.tile([128, NB, COLS], F32, tag="o_t")
            t_t = loads.tile([128, NB, COLS], F32, tag="t_t")
            nc.sync.dma_start(o_t[:], o_v[g])
            nc.sync.dma_start(t_t[:], t_v[g])

            for half_pair in range(NB // 2):
                psumP = psP.tile([128, 1280], F32)
                pr = g * (NB // 2) + half_pair
                for half in range(2):
                    b = half_pair * 2 + half
                    # d = o - t, phase-major columns: d[:, ph, j] =
                    # (o-t)[:, 4j+ph]  (bf16 out, one 3D-AP op)
                    ov = o_t[:, b, :].rearrange("p (j f) -> p f j", f=4)
                    tv = t_t[:, b, :].rearrange("p (j f) -> p f j", f=4)
                    d = bands.tile([128, 4, 128], BF16, tag="d")
                    nc.vector.tensor_sub(d[:], ov, tv)
                    # cs = [csE | csO] = col pair-sums; cd likewise
                    # (phases {0,1} and {2,3} combine; contiguous bf16 2x)
                    d4 = d[:]
                    cs = bands.tile([128, 2, 128], BF16, tag="cs")
                    cd = bands.tile([128, 2, 128], BF16, tag="cd")
                    nc.vector.tensor_add(cs[:], d4[:, 0:4:2, :], d4[:, 1:4:2, :])
                    nc.vector.tensor_sub(cd[:], d4[:, 1:4:2, :], d4[:, 0:4:2, :])
                    # level-1 bands into this pair's PSUM region
                    cb = half * 512
                    mm(psumP[:, cb:cb + 256], lhsT=w1q_t[:],
                       rhs=cs[:].rearrange("p a b -> p (a b)"),
                       start=True, stop=True)
                    mm(psumP[:, cb + 256:cb + 512], lhsT=w1_t[:],
                       rhs=cd[:].rearrange("p a b -> p (a b)"),
                       start=True, stop=True)
                    # level-2 column combines (contiguous halves of cs)
                    cs2 = bands.tile([128, 128], BF16, tag="cs2")
                    cd2 = bands.tile([128, 128], BF16, tag="cd2")
                    nc.vector.tensor_add(cs2[:], cs[:, 0, :], cs[:, 1, :])
                    nc.vector.tensor_sub(cd2[:], cs[:, 1, :], cs[:, 0, :])
                    # level-2 bands: rows [HL2; HH2] and [LH2; zeros]
                    prow = half * 64
                    mm(psumP[prow:prow + 64, 1024:1152], lhsT=w24_t[:],
                       rhs=cd2[:], start=True, stop=True)
                    mm(psumP[prow:prow + 64, 1152:1280], lhsT=w24dp_t[:],
                       rhs=cs2[:], start=True, stop=True)

                # One fused |.| + per-partition sum over all six blocks.
                ab = absout.tile([128, 1280], BF16, tag="ab")
                nc.scalar.activation(ab[:], psumP[:, 0:1280], ABS,
                                     accum_out=acc[:, pr:pr + 1])

        res_t = accp.tile([128, 4], F32)
        nc.vector.memset(res_t[:], 0.0)
        nc.vector.tensor_reduce(res_t[:, 0:1], acc[:], axis=X, op=ADD)
        nc.sync.dma_start(res_d, res_t[:])

    nc.compile()
    return nc


def _get_bass():
    if "nc" not in _CACHE:
        _CACHE["nc"] = _build_bass()
    return _CACHE["nc"]


def _numpy_reference(output, target):
    """Full-precision fallback (only for the never-hit mixed-normalize case)."""
    o = output.astype(np.float64)
    t = target.astype(np.float64)
    if o.min() < 0:
        o = (o + 1.0) * 0.5
    if t.min() < 0:
        t = (t + 1.0) * 0.5

    def dwt(x):
        a = x[:, :, 0::2, 0::2]
        b = x[:, :, 0::2, 1::2]
        c = x[:, :, 1::2, 0::2]
        d = x[:, :, 1::2, 1::2]
        return (0.5 * (a + b + c + d), 0.5 * (-a - b + c + d),
                0.5 * (-a + b - c + d), 0.5 * (a - b - c + d))

    ll_o, lh_o, hl_o, hh_o = dwt(o)
    ll_t, lh_t, hl_t, hh_t = dwt(t)
    tot = (np.abs(lh_o - lh_t).mean() + np.abs(hl_o - hl_t).mean()
           + np.abs(hh_o - hh_t).mean() + 0.1 * np.abs(ll_o - ll_t).mean())
    _, lh2_o, hl2_o, hh2_o = dwt(ll_o)
    _, lh2_t, hl2_t, hh2_t = dwt(ll_t)
    tot += 0.5 * (np.abs(lh2_o - lh2_t).mean() + np.abs(hl2_o - hl2_t).mean()
                  + np.abs(hh2_o - hh2_t).mean())
    return np.float32(tot)


def _run_device(o, t, trace=False):
    """Shard [32,3,512,512] f32 arrays over 8 cores and run the Bass NEFF."""
    from concourse.bass_utils import run_bass_kernel_spmd

    nc = _get_bass()
    w1q, w1, w24, w24dp = _make_weights()
    in_maps = []
    for c in range(N_CORES):
        sl = slice(c * B_PER_CORE, (c + 1) * B_PER_CORE)
        in_maps.append({
            "o": o[sl].reshape(ROWS, COLS),
            "t": t[sl].reshape(ROWS, COLS),
            "w1q": w1q, "w1": w1, "w24": w24, "w24dp": w24dp,
        })
    res = run_bass_kernel_spmd(nc, in_maps, core_ids=list(range(N_CORES)),
                               trace=trace)
    _CACHE["last_result"] = res
    return res


def combine(results, both_norm=True):
    """Combine per-core [128, 4] abs-sum tensors into the scalar loss."""
    m = 0.0
    for r in results:
        m += r[:, 0].astype(np.float64).sum()
    n1 = float(B * C * (H // 2) * (W // 2))
    scale = 4.0 * n1 if both_norm else 2.0 * n1
    return np.float32(m / scale)


def kernel(output, target):
    o = np.ascontiguousarray(np.asarray(output, dtype=np.float32))
    t = np.ascontiguousarray(np.asarray(target, dtype=np.float32))
    o_norm = bool(o.min() < 0.0)
    t_norm = bool(t.min() < 0.0)
    if o_norm != t_norm:
        # Normalization applied to only one input: the difference is no
        # longer a pure scale of o - t.  Practically unreachable for the
        # randn inputs this problem uses.
        return _numpy_reference(o, t)

    results = [r["res"] for r in _run_device(o, t).results]
    return combine(results, both_norm=o_norm)
